# revision 2
# baseline (speedup 1.0000x reference)
"""Conv4d (2,16,24,24,24,24) x (16,16,3,3,3,3) stride1 pad1 -> (2,16,24,24,24,24).

Strategy: W-dimension Toeplitz-packed implicit GEMM on the TensorEngine.

  - Shard over 8 cores: (batch n in {0,1}) x (U in 4 chunks of 6).
  - Per core, host packs the padded input slice into SBUF-friendly layout:
      x_dram[u', (ci*8+wi), (v'*26*4 + h'*4 + b)]  shape (8, 128, 2704)
    where w' = 6*b + wi (w-window packing: 8 input w-positions per block of
    6 output w-positions, replication 8/6).
  - Stationary operand per (du,dv,dh) tap: banded Toeplitz block
      T[(ci*8+wi), (co*6+wo)] = W[co,ci,du,dv,dh,wi-wo]  (0 <= wi-wo < 3)
    so one matmul contracts K=128=(ci,wi) and produces M=96=(co,wo).
  - 27 taps PSUM-accumulate; (du,dv,dh) shifts are pure AP offsets into the
    padded (u',v',h') free dims.  float32r matmuls (1 cycle/row, N=480).
  - Epilogue: ScalarE Identity with per-partition bias, DMA out packed;
    host unshuffles (co,wo,u,v,h,b) -> (co,u,v,h,w).
"""

import sys

if "/opt/trn_rl_repo" not in sys.path:
    sys.path.insert(0, "/opt/trn_rl_repo")

import numpy as np

import concourse.bass as bass
import concourse.mybir as mybir
import concourse.tile as tile
from concourse import bacc
from concourse.bass_utils import run_bass_kernel_spmd

C = 16          # C_in = C_out
KS = 3          # kernel size per spatial dim
S = 24          # spatial extent per dim
SP = S + 2      # padded extent
NB = 4          # w blocks per row
BW = 6          # output w positions per block
WW = 8          # input w positions per block (window)
UCORE = 6       # output U slice per core
USLAB = UCORE + 2
N_CORES = 8
FREE = SP * SP * NB        # per-slab free size = 26*26*4 = 2704
NTAPS = KS * KS * KS       # 27
M_OUT = C * BW             # 96
K_IN = C * WW              # 128
VCHUNKS = [(0, 5), (5, 5), (10, 5), (15, 5), (20, 4)]
OUT_FREE = UCORE * S * S * NB   # 6*24*24*4 = 13824

_cache = {}


def _build_nc():
    if "nc" in _cache:
        return _cache["nc"]
    f32 = mybir.dt.float32
    f32r = mybir.dt.float32r
    nc = bacc.Bacc("TRN2", target_bir_lowering=False, debug=False,
                   num_devices=N_CORES)
    x_dram = nc.dram_tensor("x", [USLAB, K_IN, FREE], f32r, kind="ExternalInput")
    w_dram = nc.dram_tensor("w", [K_IN, NTAPS * M_OUT], f32r, kind="ExternalInput")
    b_dram = nc.dram_tensor("b", [K_IN, 1], f32, kind="ExternalInput")
    o_dram = nc.dram_tensor("out", [M_OUT, OUT_FREE], f32, kind="ExternalOutput")

    with tile.TileContext(nc) as tc:
        with (
            tc.tile_pool(name="xp", bufs=1) as xp,
            tc.tile_pool(name="wp", bufs=1) as wp,
            tc.tile_pool(name="bp", bufs=1) as bp,
            tc.tile_pool(name="op", bufs=4) as op,
            tc.tile_pool(name="ps", bufs=8, space="PSUM") as ps,
        ):
            wt = wp.tile([K_IN, NTAPS * M_OUT], f32r)
            nc.sync.dma_start(wt[:], w_dram[:])
            bt = bp.tile([K_IN, 1], f32)
            nc.sync.dma_start(bt[:], b_dram[:])
            slabs = []
            for u in range(USLAB):
                xt = xp.tile([K_IN, FREE], f32r, tag=f"slab{u}")
                nc.sync.dma_start(xt[:], x_dram[u])
                slabs.append(xt)

            for u in range(UCORE):
                for v0, dv_len in VCHUNKS:
                    n_free = dv_len * S * NB
                    acc = ps.tile([M_OUT, n_free], f32)
                    t = 0
                    for du in range(KS):
                        xv = slabs[u + du][:].rearrange(
                            "p (v r) -> p v r", v=SP)
                        for dv in range(KS):
                            for dh in range(KS):
                                rhs = xv[:, v0 + dv:v0 + dv + dv_len,
                                         dh * NB:dh * NB + S * NB]
                                nc.tensor.matmul(
                                    acc[:],
                                    wt[:, t * M_OUT:(t + 1) * M_OUT],
                                    rhs,
                                    start=(t == 0),
                                    stop=(t == NTAPS - 1),
                                )
                                t += 1
                    ot = op.tile([M_OUT, n_free], f32, tag="ot")
                    nc.scalar.activation(
                        ot[:], acc[:],
                        mybir.ActivationFunctionType.Identity,
                        bias=bt[:M_OUT, :],
                    )
                    col = u * (S * S * NB) + v0 * (S * NB)
                    nc.sync.dma_start(o_dram[:, col:col + n_free], ot[:])

    nc.compile()
    _cache["nc"] = nc
    return nc


def _pack_weights(weight):
    w6 = np.asarray(weight, dtype=np.float32).reshape(C, C, KS, KS, KS, KS)
    # wt[ci, tap, dw, co] = w6[co, ci, du, dv, dh, dw]
    wt = w6.transpose(1, 2, 3, 4, 5, 0).reshape(C, NTAPS, KS, C)
    T = np.zeros((C, WW, NTAPS, C, BW), dtype=np.float32)
    for dw in range(KS):
        for wo in range(BW):
            T[:, wo + dw, :, :, wo] = wt[:, :, dw, :]
    return np.ascontiguousarray(T.reshape(K_IN, NTAPS * M_OUT))


def _pack_input_core(x_n, u0):
    """x_n: (16, 24, 24, 24, 24) for one batch sample; u0: output U origin."""
    xpad = np.zeros((C, USLAB, SP, SP, SP), dtype=np.float32)
    u_lo = max(0, u0 - 1)
    u_hi = min(S, u0 + UCORE + 1)
    xpad[:, u_lo - (u0 - 1):u_hi - (u0 - 1), 1:S + 1, 1:S + 1, 1:S + 1] = \
        x_n[:, u_lo:u_hi]
    # xpk[u', ci, wi, v', h', b] = xpad[ci, u', v', h', 6b+wi]
    xpk = np.empty((USLAB, C, WW, SP, SP, NB), dtype=np.float32)
    xt = xpad.transpose(1, 0, 2, 3, 4)   # (u', ci, v', h', w')
    for wi in range(WW):
        xpk[:, :, wi] = xt[..., wi::BW][..., :NB]
    return xpk.reshape(USLAB, K_IN, FREE)


def kernel(inputs, weight, bias):
    x = np.asarray(inputs, dtype=np.float32)
    w = np.asarray(weight, dtype=np.float32)
    b = np.asarray(bias, dtype=np.float32).reshape(C)

    nc = _build_nc()
    w_packed = _pack_weights(w)
    b_packed = np.zeros((K_IN, 1), dtype=np.float32)
    b_packed[:M_OUT, 0] = np.repeat(b, BW)

    in_maps = []
    for c in range(N_CORES):
        n, u0 = c // 4, (c % 4) * UCORE
        in_maps.append({
            "x": _pack_input_core(x[n], u0),
            "w": w_packed,
            "b": b_packed,
        })

    res = run_bass_kernel_spmd(nc, in_maps, core_ids=list(range(N_CORES)))

    out = np.empty((2, C, S, S, S, S), dtype=np.float32)
    for c in range(N_CORES):
        n, u0 = c // 4, (c % 4) * UCORE
        o = res.results[c]["out"].reshape(C, BW, UCORE, S, S, NB)
        out[n, :, u0:u0 + UCORE] = o.transpose(0, 2, 3, 4, 5, 1).reshape(
            C, UCORE, S, S, S)
    return out
